# revision 2
# baseline (speedup 1.0000x reference)
"""GSAPool pairwise-distance + mean-threshold adjacency kernel for TRN2 (v7).

dist[b,i,j] = sqrt(||x_i||^2 + ||y_j||^2 - 2 x_i.y_j), mask = dist <= mean_b(dist)

Device outputs (per core, s = sample index on the core):
  v[s, i, j]  = fp16(dist32[i, j] - avg_s)   (f32 compare quantity, rounded)
  avgs[0, s]  = f32 per-sample mean of dist
Host reconstructs dist = avg_s + v (err ~2e-4 abs) and mask = (v <= 0) which
is bit-exact vs an on-device f32 compare (fp16 rounding cannot cross zero
except within +/-3e-8 of the threshold). This removes the 1 MiB/sample u8
mask store entirely: HBM traffic = 2 MiB in + 2 MiB out per sample.

fp16 PE path; xx stays fp32 (ACT bias), yy injected as an fp16 hi+lo split
through the K=2 rank-1 matmul; mean chain fp32; rowsums fused into a Pool
pass whose fp16 output is scratch.

Scheduling (in-order engine streams; emission order is the schedule):
  iteration s: loads(s+1) | compute(s) | v-pass(s-2) | mean(s-1)

Sharding: pure data-parallel over batch b: 64 samples -> 8 cores x 8 samples.
"""

import numpy as np
from contextlib import ExitStack

import concourse.bass as bass
import concourse.tile as tile
from concourse import bacc, mybir
from concourse.bass_utils import run_bass_kernel_spmd
from concourse.masks import make_identity
import concourse.bass_isa as bass_isa

B = 64
M = 1024
N = 1024
D = 256
P = 128
MT = M // P        # 8 m-tiles
NCORES = 8
S = B // NCORES    # 8 samples per core
F32 = mybir.dt.float32
F32R = mybir.dt.float32r
F16 = mybir.dt.float16
ALU = mybir.AluOpType
ACTF = mybir.ActivationFunctionType

TR_GROUPS = [(0, 0), (1, 0), (0, 1), (1, 1)]


def build_body(ctx, tc, x_d, y_d, v_d, avgs_d, n_samples):
    nc = tc.nc

    const_pool = ctx.enter_context(tc.tile_pool(name="const", bufs=1))
    ident = const_pool.tile([P, P], F32)
    make_identity(nc, ident[:])
    ones_col16 = const_pool.tile([P, 8], F16)
    nc.gpsimd.memset(ones_col16[:], 1.0)
    ones_col32 = const_pool.tile([P, 8], F32)
    nc.gpsimd.memset(ones_col32[:], 1.0)
    ones_row2h = const_pool.tile([2, P], F16)
    nc.gpsimd.memset(ones_row2h[:, :], 1.0)
    ones_row2f = const_pool.tile([2, P], F32)
    nc.gpsimd.memset(ones_row2f[:, :], 0.0)
    nc.gpsimd.memset(ones_row2f[0:1, :], 1.0)
    zeros_bias = const_pool.tile([P, 1], F32)
    nc.gpsimd.memset(zeros_bias[:], 0.0)
    avgs_sb = const_pool.tile([1, n_samples], F32)

    nat_pool = ctx.enter_context(tc.tile_pool(name="nat", bufs=2))
    tr_pool = ctx.enter_context(tc.tile_pool(name="tr", bufs=2))
    dist_pool = ctx.enter_context(tc.tile_pool(name="dist", bufs=20))
    scr_pool = ctx.enter_context(tc.tile_pool(name="scr", bufs=2))
    v16_pool = ctx.enter_context(tc.tile_pool(name="v16", bufs=3))
    small_pool = ctx.enter_context(tc.tile_pool(name="small", bufs=4))
    psum_tr = ctx.enter_context(tc.tile_pool(name="psum_tr", bufs=2, space="PSUM"))
    psum_d2 = ctx.enter_context(tc.tile_pool(name="psum_d2", bufs=2, space="PSUM"))
    psum_sm = ctx.enter_context(tc.tile_pool(name="psum_sm", bufs=2, space="PSUM"))

    def emit_load(s, halves=False):
        y_nat = nat_pool.tile([P, MT * D], F32, tag="y_nat")
        x_nat = nat_pool.tile([P, MT * D], F32, tag="x_nat")
        # halves: first-sample pipeline fill — interleave y/x half-loads so
        # the first transpose groups can start after a half-load
        parts = ((0, 4), (4, 4)) if halves else ((0, MT),)
        for t0, tn in parts:
            for nat, dram in ((y_nat, y_d), (x_nat, x_d)):
                nc.sync.dma_start(
                    out=nat[:, t0 * D:(t0 + tn) * D].rearrange(
                        "p (t d) -> p t d", t=tn
                    ),
                    in_=dram[s, t0 * P:(t0 + tn) * P].rearrange(
                        "(t p) d -> p t d", p=P
                    ),
                )
        return x_nat, y_nat

    def emit_compute(s, x_nat, y_nat):
        # xx row norms (fp32): xx8[p, t] = sum_d x[128t+p, d]^2
        # split across DVE (scalar_tensor_tensor) and ACT to balance load
        xx8 = small_pool.tile([P, MT], F32, tag="xx8")
        for t in range(MT):
            sq_scratch = small_pool.tile([P, D], F32, tag="sq_scratch")
            xsl = x_nat[:, t * D:(t + 1) * D]
            if t < 4:
                nc.vector.scalar_tensor_tensor(
                    sq_scratch[:], xsl, 1.0, xsl,
                    ALU.mult, ALU.mult,
                    accum_out=xx8[:, t:t + 1],
                )
            else:
                nc.scalar.activation(
                    sq_scratch[:], xsl, ACTF.Square,
                    bias=zeros_bias[:, 0:1], scale=1.0,
                    accum_out=xx8[:, t:t + 1],
                )

        xTm2 = tr_pool.tile([P, 2 * M], F16, tag="xTm2")
        yT = tr_pool.tile([P, 2 * N], F16, tag="yT")
        ysq = tr_pool.tile([P, 2 * N], F16, tag="ysq")
        for kt, th in TR_GROUPS:    # y transposes
            ptry = psum_tr.tile([P, 512], F32, tag="ptr")
            for u in range(4):
                t = th * 4 + u
                nc.tensor.transpose(
                    ptry[:, u * P:(u + 1) * P],
                    y_nat[:, t * D + kt * P: t * D + kt * P + P],
                    ident[:],
                )
            sl = slice(kt * N + th * 512, kt * N + th * 512 + 512)
            nc.vector.tensor_copy(yT[:, sl], ptry[:])
            nc.vector.scalar_tensor_tensor(
                ysq[:, sl], yT[:, sl], 1.0, yT[:, sl], ALU.mult, ALU.mult
            )
        for kt, th in TR_GROUPS:    # x transposes
            ptrx = psum_tr.tile([P, 512], F32, tag="ptr")
            for u in range(4):
                t = th * 4 + u
                nc.tensor.transpose(
                    ptrx[:, u * P:(u + 1) * P],
                    x_nat[:, t * D + kt * P: t * D + kt * P + P],
                    ident[:],
                )
            sl = slice(kt * M + th * 512, kt * M + th * 512 + 512)
            nc.vector.tensor_scalar_mul(xTm2[:, sl], ptrx[:], -2.0)

        # yy row [2, N]: fp16 hi/lo split of the fp32 partition sum (DVE)
        yyrow = small_pool.tile([2, N], F16, tag="yyrow")
        pyys = []
        for nh in range(2):
            pyy = psum_sm.tile([8, 512], F32, tag="sm")
            for kt in range(2):
                nc.tensor.matmul(
                    pyy[:],
                    ones_col16[:],
                    ysq[:, kt * N + nh * 512: kt * N + nh * 512 + 512],
                    start=(kt == 0),
                    stop=(kt == 1),
                )
            sl = slice(nh * 512, (nh + 1) * 512)
            nc.vector.tensor_copy(yyrow[0:1, sl], pyy[0:1, :])
            pyys.append(pyy)
        # engines may not write APs starting at partition 1, so the lo row
        # is computed at partition 0 and DMA'd into yyrow row 1 (per half, so
        # the first rank-1 matmul unblocks as early as possible)
        yylo = small_pool.tile([1, N], F16, tag="yylo")
        for nh in range(2):
            sl = slice(nh * 512, (nh + 1) * 512)
            nc.vector.tensor_sub(yylo[0:1, sl], pyys[nh][0:1, :], yyrow[0:1, sl])
            nc.sync.dma_start(out=yyrow[1:2, sl], in_=yylo[0:1, sl])

        # main matmuls (fp16) + sqrt (fp32) with fused rowsum accumulation.
        # The first two i-tiles emit their kt matmuls up front (filling all
        # four PSUM banks) and take the yy rank-1 + sqrt afterwards, hiding
        # the yy-chain latency behind PE work.
        rs = small_pool.tile([P, MT], F32, tag="rs")
        dist_tiles = []

        def emit_ktmms(i):
            # one [P, 1024] PSUM tile spanning two banks; each matmul's out
            # AP stays within a single bank
            pd2 = psum_d2.tile([P, N], F32, tag="pd2")
            for nh in range(2):
                for kt in range(2):
                    nc.tensor.matmul(
                        pd2[:, nh * 512:(nh + 1) * 512],
                        xTm2[:, kt * M + i * P: kt * M + (i + 1) * P],
                        yT[:, kt * N + nh * 512: kt * N + nh * 512 + 512],
                        start=(kt == 0),
                        stop=False,
                    )
            return pd2

        def emit_rank1(i, nh, pd2):
            nc.tensor.matmul(
                pd2[:, nh * 512:(nh + 1) * 512],
                ones_row2h[:],
                yyrow[:, nh * 512:(nh + 1) * 512],
                start=False,
                stop=True,
            )

        def emit_sqrt(i, pd2, dt_tile):
            nc.scalar.activation(
                dt_tile[:],
                pd2[:],
                ACTF.Sqrt,
                bias=xx8[:, i:i + 1],
                scale=1.0,
                accum_out=rs[:, i:i + 1],
            )

        head = []
        for i in range(2):
            dt_tile = dist_pool.tile([P, N], F32, tag="dist")
            dist_tiles.append(dt_tile)
            head.append((i, emit_ktmms(i), dt_tile))
        for i, pd2, dt_tile in head:
            emit_rank1(i, 0, pd2)
            emit_rank1(i, 1, pd2)
            emit_sqrt(i, pd2, dt_tile)
        for i in range(2, MT):
            dt_tile = dist_pool.tile([P, N], F32, tag="dist")
            dist_tiles.append(dt_tile)
            pd2 = emit_ktmms(i)
            emit_rank1(i, 0, pd2)
            emit_rank1(i, 1, pd2)
            emit_sqrt(i, pd2, dt_tile)
        return dist_tiles, rs

    def emit_mean(s, rs):
        # mean via gpsimd partition-reduce + tiny DVE ops: no PE matmuls or
        # PSUM banks, so this can sit at the iteration head without stalls
        prs = small_pool.tile([P, MT], F32, tag="prs")
        nc.gpsimd.partition_all_reduce(prs[:], rs[:], P, bass_isa.ReduceOp.add)
        tot1 = small_pool.tile([1, 1], F32, tag="tot1")
        nc.vector.tensor_reduce(
            out=tot1[0:1, 0:1], in_=prs[0:1, :], axis=mybir.AxisListType.X,
            op=ALU.add,
        )
        avg1 = small_pool.tile([1, 1], F32, tag="avg1")
        nc.vector.tensor_scalar_mul(avg1[0:1, 0:1], tot1[0:1, 0:1],
                                    1.0 / float(M * N))
        neg1 = small_pool.tile([1, 1], F32, tag="neg1")
        nc.vector.tensor_scalar_mul(neg1[0:1, 0:1], tot1[0:1, 0:1],
                                    -1.0 / float(M * N))
        nc.vector.tensor_copy(avgs_sb[0:1, s:s + 1], avg1[0:1, 0:1])
        # materialized [P, 1] broadcasts for the v-pass operands
        avg = small_pool.tile([P, 1], F32, tag="avg")
        nc.gpsimd.partition_broadcast(avg[:], avg1[0:1, :])
        negavg = small_pool.tile([P, 1], F32, tag="negavg")
        nc.gpsimd.partition_broadcast(negavg[:], neg1[0:1, :])
        return avg, negavg

    def emit_vpass(s, dist_tiles, avg, negavg, split=False):
        """v = fp16(dist - avg); sign(v) encodes the mask. Pairs of m-tiles
        share one DMA (2 KiB lines). With split=True, half the subtracts run
        on ACT (drain tail, where ACT is otherwise idle)."""
        for u in range(MT // 2):
            vpair = v16_pool.tile([P, 2 * N], F16, tag="v16")
            for h in range(2):
                out_ap = vpair[:, h * N:(h + 1) * N]
                in_tile = dist_tiles[2 * u + h]
                if split:
                    # drain: all engines idle — spread 4 DVE / 2 ACT / 2 Pool
                    if h == 0:
                        nc.vector.tensor_scalar(
                            out_ap, in_tile[:], avg[:, 0:1], None, ALU.subtract,
                        )
                    elif u < 2:
                        nc.scalar.activation(
                            out_ap, in_tile[:], ACTF.Identity,
                            bias=negavg[:, 0:1], scale=1.0,
                        )
                    else:
                        nc.gpsimd.tensor_scalar(
                            out_ap, in_tile[:], avg[:, 0:1], None, ALU.subtract,
                        )
                    continue
                if h == 0 and u == 3:
                    nc.scalar.activation(
                        out_ap, in_tile[:], ACTF.Identity,
                        bias=negavg[:, 0:1], scale=1.0,
                    )
                elif h == 0 and u == 0:
                    # keep one on DVE; the rest go to the idle Pool engine so
                    # the DVE stream reaches the next sample's copy-outs fast
                    nc.vector.tensor_scalar(
                        out_ap, in_tile[:], avg[:, 0:1], None, ALU.subtract,
                    )
                else:
                    nc.gpsimd.tensor_scalar(
                        out_ap, in_tile[:], avg[:, 0:1], None, ALU.subtract,
                    )
            nc.sync.dma_start(
                out=v_d[s, u * 2 * P:(u + 1) * 2 * P, :].rearrange(
                    "(h p) n -> p h n", p=P
                ),
                in_=vpair.rearrange("p (h n) -> p h n", h=2),
            )

    tiles = {}
    rss = {}
    avgs = {}
    nat = {0: emit_load(0, halves=True)}
    for s in range(n_samples):
        if s + 1 < n_samples:
            nat[s + 1] = emit_load(s + 1)
        x_nat, y_nat = nat.pop(s)
        tiles[s], rss[s] = emit_compute(s, x_nat, y_nat)
        if s - 1 >= 0:
            # tail: mean then v-pass of the previous sample; their inputs
            # resolve early in sample s, long before the engines' in-order
            # streams reach these instructions
            avg_neg = emit_mean(s - 1, rss.pop(s - 1))
            emit_vpass(s - 1, tiles.pop(s - 1), *avg_neg)
    last = n_samples - 1
    avgs[last] = emit_mean(last, rss.pop(last))
    emit_vpass(last, tiles.pop(last), *avgs.pop(last), split=True)
    nc.sync.dma_start(out=avgs_d[:, :], in_=avgs_sb[:])


def build_program(n_samples=S, num_devices=NCORES):
    nc = bacc.Bacc(
        "TRN2", target_bir_lowering=False, debug=False, num_devices=num_devices
    )
    x_d = nc.dram_tensor("x", [n_samples, M, D], F32, kind="ExternalInput").ap()
    y_d = nc.dram_tensor("y", [n_samples, N, D], F32, kind="ExternalInput").ap()
    v_d = nc.dram_tensor("v", [n_samples, M, N], F16, kind="ExternalOutput").ap()
    avgs_d = nc.dram_tensor("avgs", [1, n_samples], F32, kind="ExternalOutput").ap()
    with tile.TileContext(nc) as tc:
        with ExitStack() as ctx:
            build_body(ctx, tc, x_d, y_d, v_d, avgs_d, n_samples)
    nc.compile()
    return nc


_nc_cache = None


def _get_nc():
    global _nc_cache
    if _nc_cache is None:
        _nc_cache = build_program()
    return _nc_cache


def kernel(x, y):
    x = np.ascontiguousarray(np.asarray(x), dtype=np.float32).reshape(B, M, D)
    y = np.ascontiguousarray(np.asarray(y), dtype=np.float32).reshape(B, N, D)
    nc = _get_nc()
    in_maps = [
        {
            "x": np.ascontiguousarray(x[c * S:(c + 1) * S]),
            "y": np.ascontiguousarray(y[c * S:(c + 1) * S]),
        }
        for c in range(NCORES)
    ]
    res = run_bass_kernel_spmd(nc, in_maps, list(range(NCORES)))
    dist = np.empty((B, M, N), np.float32)
    mask = np.empty((B, M, N), bool)
    for c in range(NCORES):
        v = np.asarray(res.results[c]["v"])
        avgs = np.asarray(res.results[c]["avgs"], np.float32).reshape(S)
        sl = slice(c * S, (c + 1) * S)
        mask[sl] = v <= 0
        dist[sl] = v.astype(np.float32) + avgs[:, None, None]
    return dist, mask


# revision 4
# speedup vs baseline: 2.3144x; 2.3144x over previous
"""GSAPool pairwise-distance + mean-threshold adjacency kernel for TRN2 (v7).

dist[b,i,j] = sqrt(||x_i||^2 + ||y_j||^2 - 2 x_i.y_j), mask = dist <= mean_b(dist)

Device outputs (per core, s = sample index on the core):
  v[s, i, j]  = fp16(dist32[i, j] - avg_s)   (f32 compare quantity, rounded)
  avgs[0, s]  = f32 per-sample mean of dist
Host reconstructs dist = avg_s + v (err ~2e-4 abs) and mask = (v <= 0) which
is bit-exact vs an on-device f32 compare (fp16 rounding cannot cross zero
except within +/-3e-8 of the threshold). This removes the 1 MiB/sample u8
mask store entirely: HBM traffic = 2 MiB in + 2 MiB out per sample.

fp16 PE path; xx stays fp32 (ACT bias), yy injected as an fp16 hi+lo split
through the K=2 rank-1 matmul; mean chain fp32; rowsums fused into a Pool
pass whose fp16 output is scratch.

Scheduling (in-order engine streams; emission order is the schedule):
  iteration s: loads(s+1) | compute(s) | v-pass(s-2) | mean(s-1)

Sharding: pure data-parallel over batch b: 64 samples -> 8 cores x 8 samples.
"""

import numpy as np
from contextlib import ExitStack

import concourse.bass as bass
import concourse.tile as tile
from concourse import bacc, mybir
from concourse.bass_utils import run_bass_kernel_spmd
from concourse.masks import make_identity
import concourse.bass_isa as bass_isa

B = 64
M = 1024
N = 1024
D = 256
P = 128
MT = M // P        # 8 m-tiles
NCORES = 8
S = B // NCORES    # 8 samples per core
F32 = mybir.dt.float32
F32R = mybir.dt.float32r
F16 = mybir.dt.float16
ALU = mybir.AluOpType
ACTF = mybir.ActivationFunctionType

TR_GROUPS = [(0, 0), (1, 0), (0, 1), (1, 1)]


def build_body(ctx, tc, x_d, y_d, v_d, avgs_d, n_samples):
    nc = tc.nc

    const_pool = ctx.enter_context(tc.tile_pool(name="const", bufs=1))
    ident = const_pool.tile([P, P], F32)
    make_identity(nc, ident[:])
    ones_col16 = const_pool.tile([P, 8], F16)
    nc.gpsimd.memset(ones_col16[:], 1.0)
    ones_col32 = const_pool.tile([P, 8], F32)
    nc.gpsimd.memset(ones_col32[:], 1.0)
    ones_row2h = const_pool.tile([2, P], F16)
    nc.gpsimd.memset(ones_row2h[:, :], 1.0)
    ones_row2f = const_pool.tile([2, P], F32)
    nc.gpsimd.memset(ones_row2f[:, :], 0.0)
    nc.gpsimd.memset(ones_row2f[0:1, :], 1.0)
    zeros_bias = const_pool.tile([P, 1], F32)
    nc.gpsimd.memset(zeros_bias[:], 0.0)
    avgs_sb = const_pool.tile([1, n_samples], F32)

    nat_pool = ctx.enter_context(tc.tile_pool(name="nat", bufs=2))
    tr_pool = ctx.enter_context(tc.tile_pool(name="tr", bufs=2))
    dist_pool = ctx.enter_context(tc.tile_pool(name="dist", bufs=20))
    scr_pool = ctx.enter_context(tc.tile_pool(name="scr", bufs=2))
    v16_pool = ctx.enter_context(tc.tile_pool(name="v16", bufs=3))
    small_pool = ctx.enter_context(tc.tile_pool(name="small", bufs=4))
    psum_tr = ctx.enter_context(tc.tile_pool(name="psum_tr", bufs=2, space="PSUM"))
    psum_d2 = ctx.enter_context(tc.tile_pool(name="psum_d2", bufs=2, space="PSUM"))
    psum_sm = ctx.enter_context(tc.tile_pool(name="psum_sm", bufs=2, space="PSUM"))

    def emit_load(s, halves=False):
        y_nat = nat_pool.tile([P, MT * D], F32, tag="y_nat")
        x_nat = nat_pool.tile([P, MT * D], F32, tag="x_nat")
        # halves: first-sample pipeline fill — interleave y/x half-loads so
        # the first transpose groups can start after a half-load
        parts = ((0, 4), (4, 4)) if halves else ((0, MT),)
        for t0, tn in parts:
            for nat, dram in ((y_nat, y_d), (x_nat, x_d)):
                nc.sync.dma_start(
                    out=nat[:, t0 * D:(t0 + tn) * D].rearrange(
                        "p (t d) -> p t d", t=tn
                    ),
                    in_=dram[s, t0 * P:(t0 + tn) * P].rearrange(
                        "(t p) d -> p t d", p=P
                    ),
                )
        return x_nat, y_nat

    def emit_compute(s, x_nat, y_nat):
        # xx row norms (fp32): xx8[p, t] = sum_d x[128t+p, d]^2
        # split across DVE (scalar_tensor_tensor) and ACT to balance load
        xx8 = small_pool.tile([P, MT], F32, tag="xx8")
        for t in range(MT):
            sq_scratch = small_pool.tile([P, D], F32, tag="sq_scratch")
            xsl = x_nat[:, t * D:(t + 1) * D]
            if t < 4:
                nc.vector.scalar_tensor_tensor(
                    sq_scratch[:], xsl, 1.0, xsl,
                    ALU.mult, ALU.mult,
                    accum_out=xx8[:, t:t + 1],
                )
            else:
                nc.scalar.activation(
                    sq_scratch[:], xsl, ACTF.Square,
                    bias=zeros_bias[:, 0:1], scale=1.0,
                    accum_out=xx8[:, t:t + 1],
                )

        xTm2 = tr_pool.tile([P, 2 * M], F16, tag="xTm2")
        yT = tr_pool.tile([P, 2 * N], F16, tag="yT")
        ysq = tr_pool.tile([P, 2 * N], F16, tag="ysq")
        for kt, th in TR_GROUPS:    # y transposes
            ptry = psum_tr.tile([P, 512], F32, tag="ptr")
            for u in range(4):
                t = th * 4 + u
                nc.tensor.transpose(
                    ptry[:, u * P:(u + 1) * P],
                    y_nat[:, t * D + kt * P: t * D + kt * P + P],
                    ident[:],
                )
            sl = slice(kt * N + th * 512, kt * N + th * 512 + 512)
            nc.vector.tensor_copy(yT[:, sl], ptry[:])
            nc.vector.scalar_tensor_tensor(
                ysq[:, sl], yT[:, sl], 1.0, yT[:, sl], ALU.mult, ALU.mult
            )
        for kt, th in TR_GROUPS:    # x transposes
            ptrx = psum_tr.tile([P, 512], F32, tag="ptr")
            for u in range(4):
                t = th * 4 + u
                nc.tensor.transpose(
                    ptrx[:, u * P:(u + 1) * P],
                    x_nat[:, t * D + kt * P: t * D + kt * P + P],
                    ident[:],
                )
            sl = slice(kt * M + th * 512, kt * M + th * 512 + 512)
            nc.vector.tensor_scalar_mul(xTm2[:, sl], ptrx[:], -2.0)

        # yy row [2, N]: fp16 hi/lo split of the fp32 partition sum (DVE)
        yyrow = small_pool.tile([2, N], F16, tag="yyrow")
        pyys = []
        for nh in range(2):
            pyy = psum_sm.tile([8, 512], F32, tag="sm")
            for kt in range(2):
                nc.tensor.matmul(
                    pyy[:],
                    ones_col16[:],
                    ysq[:, kt * N + nh * 512: kt * N + nh * 512 + 512],
                    start=(kt == 0),
                    stop=(kt == 1),
                )
            sl = slice(nh * 512, (nh + 1) * 512)
            nc.vector.tensor_copy(yyrow[0:1, sl], pyy[0:1, :])
            pyys.append(pyy)
        # engines may not write APs starting at partition 1, so the lo row
        # is computed at partition 0 and DMA'd into yyrow row 1 (per half, so
        # the first rank-1 matmul unblocks as early as possible)
        yylo = small_pool.tile([1, N], F16, tag="yylo")
        for nh in range(2):
            sl = slice(nh * 512, (nh + 1) * 512)
            nc.vector.tensor_sub(yylo[0:1, sl], pyys[nh][0:1, :], yyrow[0:1, sl])
            nc.sync.dma_start(out=yyrow[1:2, sl], in_=yylo[0:1, sl])

        # main matmuls (fp16) + sqrt (fp32) with fused rowsum accumulation.
        # The first two i-tiles emit their kt matmuls up front (filling all
        # four PSUM banks) and take the yy rank-1 + sqrt afterwards, hiding
        # the yy-chain latency behind PE work.
        rs = small_pool.tile([P, MT], F32, tag="rs")
        dist_tiles = []

        def emit_ktmms(i):
            # one [P, 1024] PSUM tile spanning two banks; each matmul's out
            # AP stays within a single bank
            pd2 = psum_d2.tile([P, N], F32, tag="pd2")
            for nh in range(2):
                for kt in range(2):
                    nc.tensor.matmul(
                        pd2[:, nh * 512:(nh + 1) * 512],
                        xTm2[:, kt * M + i * P: kt * M + (i + 1) * P],
                        yT[:, kt * N + nh * 512: kt * N + nh * 512 + 512],
                        start=(kt == 0),
                        stop=False,
                    )
            return pd2

        def emit_rank1(i, nh, pd2):
            nc.tensor.matmul(
                pd2[:, nh * 512:(nh + 1) * 512],
                ones_row2h[:],
                yyrow[:, nh * 512:(nh + 1) * 512],
                start=False,
                stop=True,
            )

        def emit_sqrt(i, pd2, dt_tile):
            nc.scalar.activation(
                dt_tile[:],
                pd2[:],
                ACTF.Sqrt,
                bias=xx8[:, i:i + 1],
                scale=1.0,
                accum_out=rs[:, i:i + 1],
            )

        head = []
        for i in range(2):
            dt_tile = dist_pool.tile([P, N], F32, tag="dist")
            dist_tiles.append(dt_tile)
            head.append((i, emit_ktmms(i), dt_tile))
        for i, pd2, dt_tile in head:
            emit_rank1(i, 0, pd2)
            emit_rank1(i, 1, pd2)
            emit_sqrt(i, pd2, dt_tile)
        for i in range(2, MT):
            dt_tile = dist_pool.tile([P, N], F32, tag="dist")
            dist_tiles.append(dt_tile)
            pd2 = emit_ktmms(i)
            emit_rank1(i, 0, pd2)
            emit_rank1(i, 1, pd2)
            emit_sqrt(i, pd2, dt_tile)
        return dist_tiles, rs

    def emit_mean(s, rs):
        # mean via gpsimd partition-reduce + tiny DVE ops: no PE matmuls or
        # PSUM banks, so this can sit at the iteration head without stalls
        prs = small_pool.tile([P, MT], F32, tag="prs")
        nc.gpsimd.partition_all_reduce(prs[:], rs[:], P, bass_isa.ReduceOp.add)
        tot1 = small_pool.tile([1, 1], F32, tag="tot1")
        nc.vector.tensor_reduce(
            out=tot1[0:1, 0:1], in_=prs[0:1, :], axis=mybir.AxisListType.X,
            op=ALU.add,
        )
        avg1 = small_pool.tile([1, 1], F32, tag="avg1")
        nc.vector.tensor_scalar_mul(avg1[0:1, 0:1], tot1[0:1, 0:1],
                                    1.0 / float(M * N))
        neg1 = small_pool.tile([1, 1], F32, tag="neg1")
        nc.vector.tensor_scalar_mul(neg1[0:1, 0:1], tot1[0:1, 0:1],
                                    -1.0 / float(M * N))
        nc.vector.tensor_copy(avgs_sb[0:1, s:s + 1], avg1[0:1, 0:1])
        # materialized [P, 1] broadcasts for the v-pass operands
        avg = small_pool.tile([P, 1], F32, tag="avg")
        nc.gpsimd.partition_broadcast(avg[:], avg1[0:1, :])
        negavg = small_pool.tile([P, 1], F32, tag="negavg")
        nc.gpsimd.partition_broadcast(negavg[:], neg1[0:1, :])
        return avg, negavg

    def emit_vpass(s, dist_tiles, avg, negavg, split=False):
        """v = fp16(dist - avg); sign(v) encodes the mask. Pairs of m-tiles
        share one DMA (2 KiB lines). With split=True, half the subtracts run
        on ACT (drain tail, where ACT is otherwise idle)."""
        for u in range(MT // 2):
            vpair = v16_pool.tile([P, 2 * N], F16, tag="v16")
            for h in range(2):
                out_ap = vpair[:, h * N:(h + 1) * N]
                in_tile = dist_tiles[2 * u + h]
                if split:
                    # drain: all engines idle — spread 4 DVE / 2 ACT / 2 Pool
                    if h == 0:
                        nc.vector.tensor_scalar(
                            out_ap, in_tile[:], avg[:, 0:1], None, ALU.subtract,
                        )
                    elif u < 2:
                        nc.scalar.activation(
                            out_ap, in_tile[:], ACTF.Identity,
                            bias=negavg[:, 0:1], scale=1.0,
                        )
                    else:
                        nc.gpsimd.tensor_scalar(
                            out_ap, in_tile[:], avg[:, 0:1], None, ALU.subtract,
                        )
                    continue
                if h == 0 and u == 3:
                    nc.scalar.activation(
                        out_ap, in_tile[:], ACTF.Identity,
                        bias=negavg[:, 0:1], scale=1.0,
                    )
                elif h == 0 and u == 0:
                    # keep one on DVE; the rest go to the idle Pool engine so
                    # the DVE stream reaches the next sample's copy-outs fast
                    nc.vector.tensor_scalar(
                        out_ap, in_tile[:], avg[:, 0:1], None, ALU.subtract,
                    )
                else:
                    nc.gpsimd.tensor_scalar(
                        out_ap, in_tile[:], avg[:, 0:1], None, ALU.subtract,
                    )
            nc.sync.dma_start(
                out=v_d[s, u * 2 * P:(u + 1) * 2 * P, :].rearrange(
                    "(h p) n -> p h n", p=P
                ),
                in_=vpair.rearrange("p (h n) -> p h n", h=2),
            )

    tiles = {}
    rss = {}
    avgs = {}
    nat = {0: emit_load(0, halves=True)}
    for s in range(n_samples):
        x_nat, y_nat = nat.pop(s)
        tiles[s], rss[s] = emit_compute(s, x_nat, y_nat)
        if s + 1 < n_samples:
            # prefetch next sample's inputs; dispatched mid-sample so the
            # first sample's loads aren't queued behind them
            nat[s + 1] = emit_load(s + 1)
        if s - 1 >= 0:
            # tail: mean then v-pass of the previous sample; their inputs
            # resolve early in sample s, long before the engines' in-order
            # streams reach these instructions
            avg_neg = emit_mean(s - 1, rss.pop(s - 1))
            emit_vpass(s - 1, tiles.pop(s - 1), *avg_neg)
    last = n_samples - 1
    avgs[last] = emit_mean(last, rss.pop(last))
    emit_vpass(last, tiles.pop(last), *avgs.pop(last), split=True)
    nc.sync.dma_start(out=avgs_d[:, :], in_=avgs_sb[:])


def build_program(n_samples=S, num_devices=NCORES):
    nc = bacc.Bacc(
        "TRN2", target_bir_lowering=False, debug=False, num_devices=num_devices
    )
    x_d = nc.dram_tensor("x", [n_samples, M, D], F32, kind="ExternalInput").ap()
    y_d = nc.dram_tensor("y", [n_samples, N, D], F32, kind="ExternalInput").ap()
    v_d = nc.dram_tensor("v", [n_samples, M, N], F16, kind="ExternalOutput").ap()
    avgs_d = nc.dram_tensor("avgs", [1, n_samples], F32, kind="ExternalOutput").ap()
    with tile.TileContext(nc) as tc:
        with ExitStack() as ctx:
            build_body(ctx, tc, x_d, y_d, v_d, avgs_d, n_samples)
    nc.compile()
    return nc


_nc_cache = None


def _get_nc():
    global _nc_cache
    if _nc_cache is None:
        _nc_cache = build_program()
    return _nc_cache


def kernel(x, y):
    x = np.ascontiguousarray(np.asarray(x), dtype=np.float32).reshape(B, M, D)
    y = np.ascontiguousarray(np.asarray(y), dtype=np.float32).reshape(B, N, D)
    nc = _get_nc()
    in_maps = [
        {
            "x": np.ascontiguousarray(x[c * S:(c + 1) * S]),
            "y": np.ascontiguousarray(y[c * S:(c + 1) * S]),
        }
        for c in range(NCORES)
    ]
    res = run_bass_kernel_spmd(nc, in_maps, list(range(NCORES)))
    dist = np.empty((B, M, N), np.float32)
    mask = np.empty((B, M, N), bool)
    for c in range(NCORES):
        v = np.asarray(res.results[c]["v"])
        avgs = np.asarray(res.results[c]["avgs"], np.float32).reshape(S)
        sl = slice(c * S, (c + 1) * S)
        # fp16 v <= 0  ==  int16 view <= 0 (sign bit set, or +0); v is never NaN
        mask[sl] = v.view(np.int16) <= 0
        dist[sl] = v
        dist[sl] += avgs[:, None, None]
    return dist, mask


# revision 5
# speedup vs baseline: 2.3236x; 1.0040x over previous
"""GSAPool pairwise-distance + mean-threshold adjacency kernel for TRN2 (v7).

dist[b,i,j] = sqrt(||x_i||^2 + ||y_j||^2 - 2 x_i.y_j), mask = dist <= mean_b(dist)

Device outputs (per core, s = sample index on the core):
  v[s, i, j]  = fp16(dist32[i, j] - avg_s)   (f32 compare quantity, rounded)
  avgs[0, s]  = f32 per-sample mean of dist
Host reconstructs dist = avg_s + v (err ~2e-4 abs) and mask = (v <= 0) which
is bit-exact vs an on-device f32 compare (fp16 rounding cannot cross zero
except within +/-3e-8 of the threshold). This removes the 1 MiB/sample u8
mask store entirely: HBM traffic = 2 MiB in + 2 MiB out per sample.

fp16 PE path; xx stays fp32 (ACT bias), yy injected as an fp16 hi+lo split
through the K=2 rank-1 matmul; mean chain fp32; rowsums fused into a Pool
pass whose fp16 output is scratch.

Scheduling (in-order engine streams; emission order is the schedule):
  iteration s: loads(s+1) | compute(s) | v-pass(s-2) | mean(s-1)

Sharding: pure data-parallel over batch b: 64 samples -> 8 cores x 8 samples.
"""

import numpy as np
from contextlib import ExitStack

import concourse.bass as bass
import concourse.tile as tile
from concourse import bacc, mybir
from concourse.bass_utils import run_bass_kernel_spmd
from concourse.masks import make_identity
import concourse.bass_isa as bass_isa

B = 64
M = 1024
N = 1024
D = 256
P = 128
MT = M // P        # 8 m-tiles
NCORES = 8
S = B // NCORES    # 8 samples per core
F32 = mybir.dt.float32
F32R = mybir.dt.float32r
F16 = mybir.dt.float16
ALU = mybir.AluOpType
ACTF = mybir.ActivationFunctionType

TR_GROUPS = [(0, 0), (1, 0), (0, 1), (1, 1)]


def build_body(ctx, tc, x_d, y_d, v_d, avgs_d, n_samples):
    nc = tc.nc

    const_pool = ctx.enter_context(tc.tile_pool(name="const", bufs=1))
    ident = const_pool.tile([P, P], F32)
    make_identity(nc, ident[:])
    identh = const_pool.tile([P, P], F16)
    make_identity(nc, identh[:])
    ones_col16 = const_pool.tile([P, 8], F16)
    nc.gpsimd.memset(ones_col16[:], 1.0)
    ones_col32 = const_pool.tile([P, 8], F32)
    nc.gpsimd.memset(ones_col32[:], 1.0)
    ones_row2h = const_pool.tile([2, P], F16)
    nc.gpsimd.memset(ones_row2h[:, :], 1.0)
    ones_row2f = const_pool.tile([2, P], F32)
    nc.gpsimd.memset(ones_row2f[:, :], 0.0)
    nc.gpsimd.memset(ones_row2f[0:1, :], 1.0)
    zeros_bias = const_pool.tile([P, 1], F32)
    nc.gpsimd.memset(zeros_bias[:], 0.0)
    avgs_sb = const_pool.tile([1, n_samples], F32)

    nat_pool = ctx.enter_context(tc.tile_pool(name="nat", bufs=2))
    tr_pool = ctx.enter_context(tc.tile_pool(name="tr", bufs=2))
    dist_pool = ctx.enter_context(tc.tile_pool(name="dist", bufs=20))
    scr_pool = ctx.enter_context(tc.tile_pool(name="scr", bufs=2))
    v16_pool = ctx.enter_context(tc.tile_pool(name="v16", bufs=10))
    small_pool = ctx.enter_context(tc.tile_pool(name="small", bufs=4))
    psum_tr = ctx.enter_context(tc.tile_pool(name="psum_tr", bufs=2, space="PSUM"))
    psum_d2 = ctx.enter_context(tc.tile_pool(name="psum_d2", bufs=2, space="PSUM"))
    psum_sm = ctx.enter_context(tc.tile_pool(name="psum_sm", bufs=2, space="PSUM"))

    def emit_load(s, halves=False):
        y_nat = nat_pool.tile([P, MT * D], F32, tag="y_nat")
        x_nat = nat_pool.tile([P, MT * D], F32, tag="x_nat")
        xh = nat_pool.tile([P, MT * D], F16, tag="xh")
        # halves: first-sample pipeline fill — interleave y/x half-loads so
        # the first transpose groups can start after a half-load
        parts = ((0, 4), (4, 4)) if halves else ((0, MT),)
        for t0, tn in parts:
            for nat, dram in ((y_nat, y_d), (x_nat, x_d)):
                nc.sync.dma_start(
                    out=nat[:, t0 * D:(t0 + tn) * D].rearrange(
                        "p (t d) -> p t d", t=tn
                    ),
                    in_=dram[s, t0 * P:(t0 + tn) * P].rearrange(
                        "(t p) d -> p t d", p=P
                    ),
                )
        return x_nat, y_nat, xh

    def emit_xh(x_nat, xh):
        # fp16(-2x) in natural layout on the Pool engine; feeds the fp16
        # x transposes (identical rounding to the old f32 copy-out scale)
        nc.gpsimd.tensor_scalar_mul(xh[:], x_nat[:], -2.0)

    def emit_compute(s, x_nat, y_nat, xh):
        # xx row norms (fp32): xx8[p, t] = sum_d x[128t+p, d]^2
        # split across DVE (scalar_tensor_tensor) and ACT to balance load
        xx8 = small_pool.tile([P, MT], F32, tag="xx8")
        for t in range(MT):
            sq_scratch = small_pool.tile([P, D], F32, tag="sq_scratch")
            xsl = x_nat[:, t * D:(t + 1) * D]
            if t < 6:
                nc.vector.scalar_tensor_tensor(
                    sq_scratch[:], xsl, 1.0, xsl,
                    ALU.mult, ALU.mult,
                    accum_out=xx8[:, t:t + 1],
                )
            else:
                nc.scalar.activation(
                    sq_scratch[:], xsl, ACTF.Square,
                    bias=zeros_bias[:, 0:1], scale=1.0,
                    accum_out=xx8[:, t:t + 1],
                )

        xTm2 = tr_pool.tile([P, 2 * M], F16, tag="xTm2")
        yT = tr_pool.tile([P, 2 * N], F16, tag="yT")
        ysq = tr_pool.tile([P, 2 * N], F16, tag="ysq")
        for kt, th in TR_GROUPS:    # y transposes
            ptry = psum_tr.tile([P, 512], F32, tag="ptr")
            for u in range(4):
                t = th * 4 + u
                nc.tensor.transpose(
                    ptry[:, u * P:(u + 1) * P],
                    y_nat[:, t * D + kt * P: t * D + kt * P + P],
                    ident[:],
                )
            sl = slice(kt * N + th * 512, kt * N + th * 512 + 512)
            nc.vector.tensor_copy(yT[:, sl], ptry[:])
            nc.vector.scalar_tensor_tensor(
                ysq[:, sl], yT[:, sl], 1.0, yT[:, sl], ALU.mult, ALU.mult
            )
        for kt in range(2):         # x transposes (fp16, pre-scaled by -2)
            ptrx = psum_tr.tile([P, 1024], F16, tag="ptr")
            for t in range(MT):
                nc.tensor.transpose(
                    ptrx[:, t * P:(t + 1) * P],
                    xh[:, t * D + kt * P: t * D + kt * P + P],
                    identh[:],
                )
            nc.vector.tensor_copy(xTm2[:, kt * M:(kt + 1) * M], ptrx[:])

        # yy row [2, N]: fp16 hi/lo split of the fp32 partition sum (DVE)
        yyrow = small_pool.tile([2, N], F16, tag="yyrow")
        pyys = []
        for nh in range(2):
            pyy = psum_sm.tile([8, 512], F32, tag="sm")
            for kt in range(2):
                nc.tensor.matmul(
                    pyy[:],
                    ones_col16[:],
                    ysq[:, kt * N + nh * 512: kt * N + nh * 512 + 512],
                    start=(kt == 0),
                    stop=(kt == 1),
                )
            sl = slice(nh * 512, (nh + 1) * 512)
            nc.vector.tensor_copy(yyrow[0:1, sl], pyy[0:1, :])
            pyys.append(pyy)
        # engines may not write APs starting at partition 1, so the lo row
        # is computed at partition 0 and DMA'd into yyrow row 1 (per half, so
        # the first rank-1 matmul unblocks as early as possible)
        yylo = small_pool.tile([1, N], F16, tag="yylo")
        for nh in range(2):
            sl = slice(nh * 512, (nh + 1) * 512)
            nc.vector.tensor_sub(yylo[0:1, sl], pyys[nh][0:1, :], yyrow[0:1, sl])
            nc.sync.dma_start(out=yyrow[1:2, sl], in_=yylo[0:1, sl])

        # main matmuls (fp16) + sqrt (fp32) with fused rowsum accumulation.
        # The first two i-tiles emit their kt matmuls up front (filling all
        # four PSUM banks) and take the yy rank-1 + sqrt afterwards, hiding
        # the yy-chain latency behind PE work.
        rs = small_pool.tile([P, MT], F32, tag="rs")
        dist_tiles = []

        def emit_ktmms(i):
            # one [P, 1024] PSUM tile spanning two banks; each matmul's out
            # AP stays within a single bank
            pd2 = psum_d2.tile([P, N], F32, tag="pd2")
            for nh in range(2):
                for kt in range(2):
                    nc.tensor.matmul(
                        pd2[:, nh * 512:(nh + 1) * 512],
                        xTm2[:, kt * M + i * P: kt * M + (i + 1) * P],
                        yT[:, kt * N + nh * 512: kt * N + nh * 512 + 512],
                        start=(kt == 0),
                        stop=False,
                    )
            return pd2

        def emit_rank1(i, nh, pd2):
            nc.tensor.matmul(
                pd2[:, nh * 512:(nh + 1) * 512],
                ones_row2h[:],
                yyrow[:, nh * 512:(nh + 1) * 512],
                start=False,
                stop=True,
            )

        def emit_sqrt(i, pd2, dt_tile):
            nc.scalar.activation(
                dt_tile[:],
                pd2[:],
                ACTF.Sqrt,
                bias=xx8[:, i:i + 1],
                scale=1.0,
                accum_out=rs[:, i:i + 1],
            )

        head = []
        for i in range(2):
            dt_tile = dist_pool.tile([P, N], F32, tag="dist")
            dist_tiles.append(dt_tile)
            head.append((i, emit_ktmms(i), dt_tile))
        for i, pd2, dt_tile in head:
            emit_rank1(i, 0, pd2)
            emit_rank1(i, 1, pd2)
            emit_sqrt(i, pd2, dt_tile)
        for i in range(2, MT):
            dt_tile = dist_pool.tile([P, N], F32, tag="dist")
            dist_tiles.append(dt_tile)
            pd2 = emit_ktmms(i)
            emit_rank1(i, 0, pd2)
            emit_rank1(i, 1, pd2)
            emit_sqrt(i, pd2, dt_tile)
        return dist_tiles, rs

    def emit_mean(s, rs):
        # mean via gpsimd partition-reduce + tiny DVE ops: no PE matmuls or
        # PSUM banks, so this can sit at the iteration head without stalls
        prs = small_pool.tile([P, MT], F32, tag="prs")
        nc.gpsimd.partition_all_reduce(prs[:], rs[:], P, bass_isa.ReduceOp.add)
        tot1 = small_pool.tile([1, 1], F32, tag="tot1")
        nc.vector.tensor_reduce(
            out=tot1[0:1, 0:1], in_=prs[0:1, :], axis=mybir.AxisListType.X,
            op=ALU.add,
        )
        avg1 = small_pool.tile([1, 1], F32, tag="avg1")
        nc.vector.tensor_scalar_mul(avg1[0:1, 0:1], tot1[0:1, 0:1],
                                    1.0 / float(M * N))
        neg1 = small_pool.tile([1, 1], F32, tag="neg1")
        nc.vector.tensor_scalar_mul(neg1[0:1, 0:1], tot1[0:1, 0:1],
                                    -1.0 / float(M * N))
        nc.vector.tensor_copy(avgs_sb[0:1, s:s + 1], avg1[0:1, 0:1])
        # materialized [P, 1] broadcasts for the v-pass operands
        avg = small_pool.tile([P, 1], F32, tag="avg")
        nc.gpsimd.partition_broadcast(avg[:], avg1[0:1, :])
        negavg = small_pool.tile([P, 1], F32, tag="negavg")
        nc.gpsimd.partition_broadcast(negavg[:], neg1[0:1, :])
        return avg, negavg

    def emit_vpass(s, dist_tiles, avg, negavg, split=False):
        """v = fp16(dist - avg); sign(v) encodes the mask. One subtract and
        one 2 KiB-line DMA per m-tile so stores start as early as possible.
        Engine mix: steady-state keeps DVE nearly free for the next sample's
        copy-outs; the drain (split=True) staggers all three engines."""
        if split:
            engines = ["dve", "act", "pool", "dve", "act", "pool", "dve", "dve"]
        else:
            engines = ["dve", "pool", "pool", "pool", "pool", "pool", "act", "pool"]
        for t in range(MT):
            vt = v16_pool.tile([P, N], F16, tag="v16")
            eng = engines[t]
            if eng == "act":
                nc.scalar.activation(
                    vt[:], dist_tiles[t][:], ACTF.Identity,
                    bias=negavg[:, 0:1], scale=1.0,
                )
            elif eng == "pool":
                nc.gpsimd.tensor_scalar(
                    vt[:], dist_tiles[t][:], avg[:, 0:1], None, ALU.subtract,
                )
            else:
                nc.vector.tensor_scalar(
                    vt[:], dist_tiles[t][:], avg[:, 0:1], None, ALU.subtract,
                )
            nc.sync.dma_start(out=v_d[s, t * P:(t + 1) * P, :], in_=vt[:])

    tiles = {}
    rss = {}
    avgs = {}
    nat = {0: emit_load(0, halves=True)}
    emit_xh(nat[0][0], nat[0][2])
    for s in range(n_samples):
        x_nat, y_nat, xh = nat.pop(s)
        tiles[s], rss[s] = emit_compute(s, x_nat, y_nat, xh)
        if s + 1 < n_samples:
            # prefetch next sample's inputs; dispatched mid-sample so the
            # first sample's loads aren't queued behind them
            nat[s + 1] = emit_load(s + 1)
        if s - 1 >= 0:
            # tail: mean then v-pass of the previous sample; their inputs
            # resolve early in sample s, long before the engines' in-order
            # streams reach these instructions
            avg_neg = emit_mean(s - 1, rss.pop(s - 1))
            emit_vpass(s - 1, tiles.pop(s - 1), *avg_neg)
        if s + 1 < n_samples:
            # xh conversion last: its input lands mid-sample, and Pool must
            # not block on it before the mean/v-pass work
            emit_xh(nat[s + 1][0], nat[s + 1][2])
    last = n_samples - 1
    avgs[last] = emit_mean(last, rss.pop(last))
    emit_vpass(last, tiles.pop(last), *avgs.pop(last), split=True)
    nc.sync.dma_start(out=avgs_d[:, :], in_=avgs_sb[:])


def build_program(n_samples=S, num_devices=NCORES):
    nc = bacc.Bacc(
        "TRN2", target_bir_lowering=False, debug=False, num_devices=num_devices
    )
    x_d = nc.dram_tensor("x", [n_samples, M, D], F32, kind="ExternalInput").ap()
    y_d = nc.dram_tensor("y", [n_samples, N, D], F32, kind="ExternalInput").ap()
    v_d = nc.dram_tensor("v", [n_samples, M, N], F16, kind="ExternalOutput").ap()
    avgs_d = nc.dram_tensor("avgs", [1, n_samples], F32, kind="ExternalOutput").ap()
    with tile.TileContext(nc) as tc:
        with ExitStack() as ctx:
            build_body(ctx, tc, x_d, y_d, v_d, avgs_d, n_samples)
    nc.compile()
    return nc


_nc_cache = None


def _get_nc():
    global _nc_cache
    if _nc_cache is None:
        _nc_cache = build_program()
    return _nc_cache


def kernel(x, y):
    x = np.ascontiguousarray(np.asarray(x), dtype=np.float32).reshape(B, M, D)
    y = np.ascontiguousarray(np.asarray(y), dtype=np.float32).reshape(B, N, D)
    nc = _get_nc()
    in_maps = [
        {
            "x": np.ascontiguousarray(x[c * S:(c + 1) * S]),
            "y": np.ascontiguousarray(y[c * S:(c + 1) * S]),
        }
        for c in range(NCORES)
    ]
    res = run_bass_kernel_spmd(nc, in_maps, list(range(NCORES)))
    dist = np.empty((B, M, N), np.float32)
    mask = np.empty((B, M, N), bool)
    for c in range(NCORES):
        v = np.asarray(res.results[c]["v"])
        avgs = np.asarray(res.results[c]["avgs"], np.float32).reshape(S)
        sl = slice(c * S, (c + 1) * S)
        # fp16 v <= 0  ==  int16 view <= 0 (sign bit set, or +0); v is never NaN
        mask[sl] = v.view(np.int16) <= 0
        dist[sl] = v
        dist[sl] += avgs[:, None, None]
    return dist, mask


# revision 7
# speedup vs baseline: 2.4270x; 1.0445x over previous
"""GSAPool pairwise-distance + mean-threshold adjacency kernel for TRN2 (v7).

dist[b,i,j] = sqrt(||x_i||^2 + ||y_j||^2 - 2 x_i.y_j), mask = dist <= mean_b(dist)

Device outputs (per core, s = sample index on the core):
  v[s, i, j]  = fp16(dist32[i, j] - avg_s)   (f32 compare quantity, rounded)
  avgs[0, s]  = f32 per-sample mean of dist
Host reconstructs dist = avg_s + v (err ~2e-4 abs) and mask = (v <= 0) which
is bit-exact vs an on-device f32 compare (fp16 rounding cannot cross zero
except within +/-3e-8 of the threshold). This removes the 1 MiB/sample u8
mask store entirely: HBM traffic = 2 MiB in + 2 MiB out per sample.

fp16 PE path: x is pre-converted to fp16(-2x) on the Pool engine so its
transposes run at 1 cycle/row; y transposes stay fp32 (its squares feed yy
and must not double-round). xx stays fp32 (ACT bias); yy is injected as an
fp16 hi+lo split through the K=2 rank-1 matmul; rowsums ride the ACT sqrt
accumulator; the mean uses gpsimd partition reduce/broadcast (no PE/PSUM).

Scheduling (in-order engine streams; emission order is the schedule):
  iteration s: compute(s) | loads(s+1) | mean(s-1) | v-pass(s-1) | xh(s+1)

Sharding: pure data-parallel over batch b: 64 samples -> 8 cores x 8 samples.
"""

import numpy as np
from contextlib import ExitStack

import concourse.bass as bass
import concourse.tile as tile
from concourse import bacc, mybir
from concourse.bass_utils import run_bass_kernel_spmd
from concourse.masks import make_identity
import concourse.bass_isa as bass_isa

B = 64
M = 1024
N = 1024
D = 256
P = 128
MT = M // P        # 8 m-tiles
NCORES = 8
S = B // NCORES    # 8 samples per core
F32 = mybir.dt.float32
F32R = mybir.dt.float32r
F16 = mybir.dt.float16
ALU = mybir.AluOpType
ACTF = mybir.ActivationFunctionType

TR_GROUPS = [(0, 0), (1, 0), (0, 1), (1, 1)]


def build_body(ctx, tc, x_d, y_d, v_d, avgs_d, n_samples):
    nc = tc.nc

    const_pool = ctx.enter_context(tc.tile_pool(name="const", bufs=1))
    ident = const_pool.tile([P, P], F32)
    make_identity(nc, ident[:])
    identh = const_pool.tile([P, P], F16)
    make_identity(nc, identh[:])
    ones_col16 = const_pool.tile([P, 8], F16)
    nc.gpsimd.memset(ones_col16[:], 1.0)
    ones_col32 = const_pool.tile([P, 8], F32)
    nc.gpsimd.memset(ones_col32[:], 1.0)
    ones_row2h = const_pool.tile([2, P], F16)
    nc.gpsimd.memset(ones_row2h[:, :], 1.0)
    ones_row2f = const_pool.tile([2, P], F32)
    nc.gpsimd.memset(ones_row2f[:, :], 0.0)
    nc.gpsimd.memset(ones_row2f[0:1, :], 1.0)
    zeros_bias = const_pool.tile([P, 1], F32)
    nc.gpsimd.memset(zeros_bias[:], 0.0)
    avgs_sb = const_pool.tile([1, n_samples], F32)

    nat_pool = ctx.enter_context(tc.tile_pool(name="nat", bufs=2))
    tr_pool = ctx.enter_context(tc.tile_pool(name="tr", bufs=2))
    dist_pool = ctx.enter_context(tc.tile_pool(name="dist", bufs=20))
    scr_pool = ctx.enter_context(tc.tile_pool(name="scr", bufs=2))
    v16_pool = ctx.enter_context(tc.tile_pool(name="v16", bufs=10))
    small_pool = ctx.enter_context(tc.tile_pool(name="small", bufs=4))
    psum_tr = ctx.enter_context(tc.tile_pool(name="psum_tr", bufs=2, space="PSUM"))
    psum_d2 = ctx.enter_context(tc.tile_pool(name="psum_d2", bufs=2, space="PSUM"))
    psum_sm = ctx.enter_context(tc.tile_pool(name="psum_sm", bufs=2, space="PSUM"))

    def emit_load(s, halves=False):
        y_nat = nat_pool.tile([P, MT * D], F32, tag="y_nat")
        x_nat = nat_pool.tile([P, MT * D], F32, tag="x_nat")
        xh = nat_pool.tile([P, MT * D], F16, tag="xh")
        # halves: first-sample pipeline fill — interleave y/x half-loads so
        # the first transpose groups can start after a half-load
        parts = ((0, 4), (4, 4)) if halves else ((0, MT),)
        for t0, tn in parts:
            for nat, dram in ((y_nat, y_d), (x_nat, x_d)):
                nc.sync.dma_start(
                    out=nat[:, t0 * D:(t0 + tn) * D].rearrange(
                        "p (t d) -> p t d", t=tn
                    ),
                    in_=dram[s, t0 * P:(t0 + tn) * P].rearrange(
                        "(t p) d -> p t d", p=P
                    ),
                )
        return x_nat, y_nat, xh

    def emit_xh(x_nat, xh):
        # fp16(-2x) in natural layout on the Pool engine; feeds the fp16
        # x transposes (identical rounding to the old f32 copy-out scale)
        nc.gpsimd.tensor_scalar_mul(xh[:], x_nat[:], -2.0)

    def emit_compute(s, x_nat, y_nat, xh):
        # xx row norms (fp32): xx8[p, t] = sum_d x[128t+p, d]^2
        # split across DVE (scalar_tensor_tensor) and ACT to balance load
        xx8 = small_pool.tile([P, MT], F32, tag="xx8")
        for t in range(MT):
            sq_scratch = small_pool.tile([P, D], F32, tag="sq_scratch")
            xsl = x_nat[:, t * D:(t + 1) * D]
            if t < 5:
                nc.vector.scalar_tensor_tensor(
                    sq_scratch[:], xsl, 1.0, xsl,
                    ALU.mult, ALU.mult,
                    accum_out=xx8[:, t:t + 1],
                )
            else:
                nc.scalar.activation(
                    sq_scratch[:], xsl, ACTF.Square,
                    bias=zeros_bias[:, 0:1], scale=1.0,
                    accum_out=xx8[:, t:t + 1],
                )

        xTm2 = tr_pool.tile([P, 2 * M], F16, tag="xTm2")
        yT = tr_pool.tile([P, 2 * N], F16, tag="yT")
        ysq = tr_pool.tile([P, 2 * N], F16, tag="ysq")
        for kt, th in TR_GROUPS:    # y transposes
            ptry = psum_tr.tile([P, 512], F32, tag="ptr")
            for u in range(4):
                t = th * 4 + u
                nc.tensor.transpose(
                    ptry[:, u * P:(u + 1) * P],
                    y_nat[:, t * D + kt * P: t * D + kt * P + P],
                    ident[:],
                )
            sl = slice(kt * N + th * 512, kt * N + th * 512 + 512)
            nc.vector.tensor_copy(yT[:, sl], ptry[:])
            nc.vector.scalar_tensor_tensor(
                ysq[:, sl], yT[:, sl], 1.0, yT[:, sl], ALU.mult, ALU.mult
            )
        for kt in range(2):         # x transposes (fp16, pre-scaled by -2)
            ptrx = psum_tr.tile([P, 1024], F16, tag="ptr")
            for t in range(MT):
                nc.tensor.transpose(
                    ptrx[:, t * P:(t + 1) * P],
                    xh[:, t * D + kt * P: t * D + kt * P + P],
                    identh[:],
                )
            nc.vector.tensor_copy(xTm2[:, kt * M:(kt + 1) * M], ptrx[:])

        # yy row [2, N]: fp16 hi/lo split of the fp32 partition sum (DVE)
        yyrow = small_pool.tile([2, N], F16, tag="yyrow")
        pyys = []
        for nh in range(2):
            pyy = psum_sm.tile([8, 512], F32, tag="sm")
            for kt in range(2):
                nc.tensor.matmul(
                    pyy[:],
                    ones_col16[:],
                    ysq[:, kt * N + nh * 512: kt * N + nh * 512 + 512],
                    start=(kt == 0),
                    stop=(kt == 1),
                )
            sl = slice(nh * 512, (nh + 1) * 512)
            nc.vector.tensor_copy(yyrow[0:1, sl], pyy[0:1, :])
            pyys.append(pyy)
        # engines may not write APs starting at partition 1, so the lo row
        # is computed at partition 0 and DMA'd into yyrow row 1 (per half, so
        # the first rank-1 matmul unblocks as early as possible)
        yylo = small_pool.tile([1, N], F16, tag="yylo")
        for nh in range(2):
            sl = slice(nh * 512, (nh + 1) * 512)
            nc.vector.tensor_sub(yylo[0:1, sl], pyys[nh][0:1, :], yyrow[0:1, sl])
            nc.sync.dma_start(out=yyrow[1:2, sl], in_=yylo[0:1, sl])

        # main matmuls (fp16) + sqrt (fp32) with fused rowsum accumulation.
        # The first two i-tiles emit their kt matmuls up front (filling all
        # four PSUM banks) and take the yy rank-1 + sqrt afterwards, hiding
        # the yy-chain latency behind PE work.
        rs = small_pool.tile([P, MT], F32, tag="rs")
        dist_tiles = []

        def emit_ktmms(i):
            # one [P, 1024] PSUM tile spanning two banks; each matmul's out
            # AP stays within a single bank
            pd2 = psum_d2.tile([P, N], F32, tag="pd2")
            for nh in range(2):
                for kt in range(2):
                    nc.tensor.matmul(
                        pd2[:, nh * 512:(nh + 1) * 512],
                        xTm2[:, kt * M + i * P: kt * M + (i + 1) * P],
                        yT[:, kt * N + nh * 512: kt * N + nh * 512 + 512],
                        start=(kt == 0),
                        stop=False,
                    )
            return pd2

        def emit_rank1(i, nh, pd2):
            nc.tensor.matmul(
                pd2[:, nh * 512:(nh + 1) * 512],
                ones_row2h[:],
                yyrow[:, nh * 512:(nh + 1) * 512],
                start=False,
                stop=True,
            )

        def emit_sqrt(i, pd2, dt_tile):
            nc.scalar.activation(
                dt_tile[:],
                pd2[:],
                ACTF.Sqrt,
                bias=xx8[:, i:i + 1],
                scale=1.0,
                accum_out=rs[:, i:i + 1],
            )

        head = []
        for i in range(2):
            dt_tile = dist_pool.tile([P, N], F32, tag="dist")
            dist_tiles.append(dt_tile)
            head.append((i, emit_ktmms(i), dt_tile))
        for i, pd2, dt_tile in head:
            emit_rank1(i, 0, pd2)
            emit_rank1(i, 1, pd2)
            emit_sqrt(i, pd2, dt_tile)
        for i in range(2, MT):
            dt_tile = dist_pool.tile([P, N], F32, tag="dist")
            dist_tiles.append(dt_tile)
            pd2 = emit_ktmms(i)
            emit_rank1(i, 0, pd2)
            emit_rank1(i, 1, pd2)
            emit_sqrt(i, pd2, dt_tile)
        return dist_tiles, rs

    def emit_mean(s, rs):
        # mean via gpsimd partition-reduce + tiny DVE ops: no PE matmuls or
        # PSUM banks, so this can sit at the iteration head without stalls
        prs = small_pool.tile([P, MT], F32, tag="prs")
        nc.gpsimd.partition_all_reduce(prs[:], rs[:], P, bass_isa.ReduceOp.add)
        tot1 = small_pool.tile([1, 1], F32, tag="tot1")
        nc.vector.tensor_reduce(
            out=tot1[0:1, 0:1], in_=prs[0:1, :], axis=mybir.AxisListType.X,
            op=ALU.add,
        )
        avg1 = small_pool.tile([1, 1], F32, tag="avg1")
        nc.vector.tensor_scalar_mul(avg1[0:1, 0:1], tot1[0:1, 0:1],
                                    1.0 / float(M * N))
        neg1 = small_pool.tile([1, 1], F32, tag="neg1")
        nc.vector.tensor_scalar_mul(neg1[0:1, 0:1], tot1[0:1, 0:1],
                                    -1.0 / float(M * N))
        nc.vector.tensor_copy(avgs_sb[0:1, s:s + 1], avg1[0:1, 0:1])
        # materialized [P, 1] broadcasts for the v-pass operands
        avg = small_pool.tile([P, 1], F32, tag="avg")
        nc.gpsimd.partition_broadcast(avg[:], avg1[0:1, :])
        negavg = small_pool.tile([P, 1], F32, tag="negavg")
        nc.gpsimd.partition_broadcast(negavg[:], neg1[0:1, :])
        return avg, negavg

    def emit_vpass(s, dist_tiles, avg, negavg, split=False):
        """v = fp16(dist - avg); sign(v) encodes the mask. One subtract and
        one 2 KiB-line DMA per m-tile so stores start as early as possible.
        Engine mix: steady-state keeps DVE nearly free for the next sample's
        copy-outs; the drain (split=True) staggers all three engines."""
        if split:
            engines = ["dve", "act", "pool", "dve", "act", "pool", "dve", "dve"]
        else:
            engines = ["dve", "act", "pool", "dve", "pool", "pool", "act", "pool"]
        for t in range(MT):
            vt = v16_pool.tile([P, N], F16, tag="v16")
            eng = engines[t]
            if eng == "act":
                nc.scalar.activation(
                    vt[:], dist_tiles[t][:], ACTF.Identity,
                    bias=negavg[:, 0:1], scale=1.0,
                )
            elif eng == "pool":
                nc.gpsimd.tensor_scalar(
                    vt[:], dist_tiles[t][:], avg[:, 0:1], None, ALU.subtract,
                )
            else:
                nc.vector.tensor_scalar(
                    vt[:], dist_tiles[t][:], avg[:, 0:1], None, ALU.subtract,
                )
            nc.sync.dma_start(out=v_d[s, t * P:(t + 1) * P, :], in_=vt[:])

    tiles = {}
    rss = {}
    avgs = {}
    nat = {0: emit_load(0, halves=True)}
    emit_xh(nat[0][0], nat[0][2])
    for s in range(n_samples):
        x_nat, y_nat, xh = nat.pop(s)
        tiles[s], rss[s] = emit_compute(s, x_nat, y_nat, xh)
        if s + 1 < n_samples:
            # prefetch next sample's inputs; dispatched mid-sample so the
            # first sample's loads aren't queued behind them
            nat[s + 1] = emit_load(s + 1)
        if s - 1 >= 0:
            # tail: mean then v-pass of the previous sample; their inputs
            # resolve early in sample s, long before the engines' in-order
            # streams reach these instructions
            avg_neg = emit_mean(s - 1, rss.pop(s - 1))
            emit_vpass(s - 1, tiles.pop(s - 1), *avg_neg)
        if s + 1 < n_samples:
            # xh conversion last: its input lands mid-sample, and Pool must
            # not block on it before the mean/v-pass work
            emit_xh(nat[s + 1][0], nat[s + 1][2])
    last = n_samples - 1
    avgs[last] = emit_mean(last, rss.pop(last))
    emit_vpass(last, tiles.pop(last), *avgs.pop(last), split=True)
    nc.sync.dma_start(out=avgs_d[:, :], in_=avgs_sb[:])


def build_program(n_samples=S, num_devices=NCORES):
    nc = bacc.Bacc(
        "TRN2", target_bir_lowering=False, debug=False, num_devices=num_devices
    )
    x_d = nc.dram_tensor("x", [n_samples, M, D], F32, kind="ExternalInput").ap()
    y_d = nc.dram_tensor("y", [n_samples, N, D], F32, kind="ExternalInput").ap()
    v_d = nc.dram_tensor("v", [n_samples, M, N], F16, kind="ExternalOutput").ap()
    avgs_d = nc.dram_tensor("avgs", [1, n_samples], F32, kind="ExternalOutput").ap()
    with tile.TileContext(nc) as tc:
        with ExitStack() as ctx:
            build_body(ctx, tc, x_d, y_d, v_d, avgs_d, n_samples)
    nc.compile()
    return nc


_nc_cache = None


def _get_nc():
    global _nc_cache
    if _nc_cache is None:
        _nc_cache = build_program()
    return _nc_cache


def kernel(x, y):
    x = np.ascontiguousarray(np.asarray(x), dtype=np.float32).reshape(B, M, D)
    y = np.ascontiguousarray(np.asarray(y), dtype=np.float32).reshape(B, N, D)
    nc = _get_nc()
    in_maps = [
        {
            "x": np.ascontiguousarray(x[c * S:(c + 1) * S]),
            "y": np.ascontiguousarray(y[c * S:(c + 1) * S]),
        }
        for c in range(NCORES)
    ]
    res = run_bass_kernel_spmd(nc, in_maps, list(range(NCORES)))
    dist = np.empty((B, M, N), np.float32)
    mask = np.empty((B, M, N), bool)
    for c in range(NCORES):
        v = np.asarray(res.results[c]["v"])
        avgs = np.asarray(res.results[c]["avgs"], np.float32).reshape(S)
        sl = slice(c * S, (c + 1) * S)
        # fp16 v <= 0  ==  int16 view <= 0 (sign bit set, or +0); v is never NaN
        mask[sl] = v.view(np.int16) <= 0
        dist[sl] = v
        dist[sl] += avgs[:, None, None]
    return dist, mask
